# revision 2
# baseline (speedup 1.0000x reference)
"""ANI-style species-routed MLP (MoE routing) on 8 TRN2 NeuronCores.

Strategy:
- Data-parallel over molecules: core c handles molecules [128c, 128(c+1)).
- Host groups each core's 6144 atoms by species (counting sort), pads each
  species bucket to a shared capacity, and ships the aev feature-major
  (transposed) so features sit on SBUF partitions.
- Device computes, per species segment, the dense 4-layer MLP chain with
  f32r matmuls (full PE rate), CELU via one exact trick:
      celu(x) + 0.1 = min(0.1*exp(10x), 0.1) + relu(x)
  The +0.1 offset is folded into the next layer's bias on the host
  (beta = b - 0.1 * rowsum(W)); the 0.1 output scale of the exp is folded
  into the activation bias (ln(0.1)).
- Final per-molecule reduction on host (cheap), including the per-species
  output bias b4 - 0.1*rowsum(W4).
"""
import os
import sys

sys.path.insert(0, "/opt/trn_rl_repo")

from contextlib import ExitStack

import numpy as np

import concourse.bass as bass
import concourse.mybir as mybir
import concourse.tile as tile
from concourse import bacc
from concourse.bass_utils import run_bass_kernel_spmd

F32 = mybir.dt.float32
F32R = mybir.dt.float32r
AF = mybir.ActivationFunctionType
ALU = mybir.AluOpType

B, A, F = 1024, 48, 384
S = 7
H = [384, 256, 192, 160]  # layer input sizes; outputs 256,192,160,1
NCORES = 8
BM = B // NCORES  # molecules per core
ALPHA = 0.1
LN_ALPHA = float(np.log(ALPHA))

_CACHE = {}
LAST_EXEC_NS = None


def _tile_sizes(cap):
    """Split a species capacity into matmul tiles (<=512 each)."""
    if cap == 0:
        return []
    t = -(-cap // 512)
    base = ((-(-cap // t) + 31) // 32) * 32
    sizes = [base] * (t - 1) + [cap - base * (t - 1)]
    assert all(0 < x <= 512 for x in sizes) and sum(sizes) == cap
    return sizes


def _build(caps):
    """Build the SPMD Bass graph for per-species capacities `caps`."""
    a_pad = sum(caps)
    nc = bacc.Bacc()

    xt_d = nc.declare_dram_parameter("xt", [F, a_pad], F32, isOutput=False)
    w1_d = nc.declare_dram_parameter("w1t", [S, 384, 256], F32, isOutput=False)
    w2_d = nc.declare_dram_parameter("w2t", [S, 256, 192], F32, isOutput=False)
    w3_d = nc.declare_dram_parameter("w3t", [S, 256, 160], F32, isOutput=False)
    w4_d = nc.declare_dram_parameter("w4t", [S, 256, 1], F32, isOutput=False)
    bias_d = {}
    for l in (1, 2, 3):
        for kind in ("x", "r"):
            bias_d[(l, kind)] = nc.declare_dram_parameter(
                f"b{kind}{l}", [S, 2, 128], F32, isOutput=False
            )
    en_d = nc.declare_dram_parameter("energy", [1, a_pad], F32, isOutput=True)

    nt_max = max((max(_tile_sizes(c)) for c in caps if c), default=512)

    # (K-chunks, M-chunks) per layer; M chunks given as (offset, width)
    l1_m = [(0, 128), (128, 128)]
    l2_m = [(0, 128), (128, 64)]
    l3_m = [(0, 128), (128, 32)]
    l2_k = [(0, 128), (128, 128)]
    l3_k = [(0, 128), (128, 64)]  # K=192 over u2 chunks
    l4_k = [(0, 128), (128, 32)]  # K=160 over u3 chunks

    with tile.TileContext(nc) as tc, ExitStack() as ctx:
        wpool = ctx.enter_context(tc.tile_pool(name="weights", bufs=1))
        pool = ctx.enter_context(tc.tile_pool(name="work", bufs=1))
        psum = ctx.enter_context(tc.tile_pool(name="psum", bufs=1, space="PSUM"))

        w1s, w2s, w3s, w4s, bsb = [], [], [], [], {}
        for s in range(S):
            w1 = wpool.tile([128, 3, 256], F32R, tag=f"w1_{s}")
            nc.sync.dma_start(
                w1[:], w1_d.ap()[s].bitcast(F32R).rearrange("(c p) m -> p c m", p=128)
            )
            w2 = wpool.tile([128, 2, 192], F32R, tag=f"w2_{s}")
            nc.sync.dma_start(
                w2[:], w2_d.ap()[s].bitcast(F32R).rearrange("(c p) m -> p c m", p=128)
            )
            w3 = wpool.tile([128, 2, 160], F32R, tag=f"w3_{s}")
            nc.sync.dma_start(
                w3[:], w3_d.ap()[s].bitcast(F32R).rearrange("(c p) m -> p c m", p=128)
            )
            w4 = wpool.tile([128, 2, 1], F32R, tag=f"w4_{s}")
            nc.sync.dma_start(
                w4[:], w4_d.ap()[s].bitcast(F32R).rearrange("(c p) m -> p c m", p=128)
            )
            w1s.append(w1), w2s.append(w2), w3s.append(w3), w4s.append(w4)
            for l in (1, 2, 3):
                for kind in ("x", "r"):
                    bt = wpool.tile([128, 2], F32, tag=f"b{kind}{l}_{s}")
                    nc.sync.dma_start(
                        bt[:], bias_d[(l, kind)].ap()[s].rearrange("c p -> p c")
                    )
                    bsb[(s, l, kind)] = bt

        chunk_idx = 0  # global celu-chunk counter for ACT/DVE relu balancing

        def celu(z, u_out, s, l, m):
            """u_out[:p, :] = celu(z + beta)+0.1 where p=partitions of z."""
            nonlocal chunk_idx
            p = z.shape[0]
            nt = z.shape[-1]
            bx = bsb[(s, l, "x")][:p, m : m + 1]
            br = bsb[(s, l, "r")][:p, m : m + 1]
            e = pool.tile([128, nt_max], F32, tag="e")
            nc.scalar.activation(e[:p, :nt], z[:], AF.Exp, bias=bx, scale=10.0)
            r = pool.tile([128, nt_max], F32, tag="r")
            if chunk_idx % 2 == 0:
                nc.vector.tensor_scalar(
                    r[:p, :nt], z[:], br, 0.0, op0=ALU.add, op1=ALU.max
                )
            else:
                nc.scalar.activation(r[:p, :nt], z[:], AF.Relu, bias=br, scale=1.0)
            chunk_idx += 1
            nc.vector.scalar_tensor_tensor(
                u_out[:], e[:p, :nt], ALPHA, r[:p, :nt], op0=ALU.min, op1=ALU.add
            )

        off = 0
        for s in range(S):
            for nt in _tile_sizes(caps[s]):
                x = pool.tile([128, 3, nt_max], F32R, tag="x")
                nc.sync.dma_start(
                    x[:, :, :nt],
                    xt_d.ap()[:, off : off + nt]
                    .bitcast(F32R)
                    .rearrange("(c p) n -> p c n", p=128),
                )
                # L1: 384 -> 256
                u1 = pool.tile([128, 2, nt], F32R, tag="u1")
                for mi, (mo, mw) in enumerate(l1_m):
                    z = psum.tile([128, nt], F32, tag="z1")
                    for k in range(3):
                        nc.tensor.matmul(
                            z[:mw, :],
                            w1s[s][:, k, mo : mo + mw],
                            x[:, k, :nt],
                            start=(k == 0),
                            stop=(k == 2),
                        )
                    celu(z[:mw, :], u1[:mw, mi, :], s, 1, mi)
                # L2: 256 -> 192
                u2 = pool.tile([128, 2, nt], F32R, tag="u2")
                for mi, (mo, mw) in enumerate(l2_m):
                    z = psum.tile([128, nt], F32, tag="z2")
                    for ki, (ko, kw) in enumerate(l2_k):
                        nc.tensor.matmul(
                            z[:mw, :],
                            w2s[s][:kw, ki, mo : mo + mw],
                            u1[:kw, ki, :],
                            start=(ki == 0),
                            stop=(ki == 1),
                        )
                    celu(z[:mw, :], u2[:mw, mi, :], s, 2, mi)
                # L3: 192 -> 160
                u3 = pool.tile([128, 2, nt], F32R, tag="u3")
                for mi, (mo, mw) in enumerate(l3_m):
                    z = psum.tile([128, nt], F32, tag="z3")
                    for ki, (ko, kw) in enumerate(l3_k):
                        nc.tensor.matmul(
                            z[:mw, :],
                            w3s[s][:kw, ki, mo : mo + mw],
                            u2[:kw, ki, :],
                            start=(ki == 0),
                            stop=(ki == 1),
                        )
                    celu(z[:mw, :], u3[:mw, mi, :], s, 3, mi)
                # L4: 160 -> 1 (no bias, no activation; host adds ec[s])
                z4 = psum.tile([1, nt], F32, tag="z4")
                for ki, (ko, kw) in enumerate(l4_k):
                    nc.tensor.matmul(
                        z4[:],
                        w4s[s][:kw, ki, 0:1],
                        u3[:kw, ki, :],
                        start=(ki == 0),
                        stop=(ki == 1),
                    )
                en_sb = pool.tile([1, nt_max], F32, tag="en")
                if chunk_idx % 2 == 0:
                    nc.vector.tensor_copy(en_sb[:, :nt], z4[:])
                else:
                    nc.scalar.activation(en_sb[:, :nt], z4[:], AF.Copy)
                nc.sync.dma_start(en_d.ap()[:, off : off + nt], en_sb[:, :nt])
                off += nt

    nc.compile()
    return nc


def _prep_weights(W1, b1, W2, b2, W3, b3, W4, b4):
    beta1 = b1
    beta2 = b2 - ALPHA * W2.sum(axis=2)
    beta3 = b3 - ALPHA * W3.sum(axis=2)
    ec = (b4[:, 0] - ALPHA * W4[:, 0, :].sum(axis=1)).astype(np.float32)

    def pad_k(wt, k_to):  # wt [S, K, M] -> [S, k_to, M]
        out = np.zeros((S, k_to, wt.shape[2]), np.float32)
        out[:, : wt.shape[1]] = wt
        return out

    w1t = np.ascontiguousarray(W1.transpose(0, 2, 1))  # [S, 384, 256]
    w2t = np.ascontiguousarray(W2.transpose(0, 2, 1))  # [S, 256, 192]
    w3t = pad_k(W3.transpose(0, 2, 1), 256)  # [S, 256, 160]
    w4t = pad_k(W4.transpose(0, 2, 1), 256)  # [S, 256, 1]

    def bias_pair(beta):  # [S, M] -> exp-bias, relu-bias as [S, 2, 128]
        m = beta.shape[1]
        bx = np.zeros((S, 256), np.float32)
        br = np.zeros((S, 256), np.float32)
        bx[:, :m] = 10.0 * beta + LN_ALPHA
        br[:, :m] = beta
        return bx.reshape(S, 2, 128), br.reshape(S, 2, 128)

    bx1, br1 = bias_pair(beta1)
    bx2, br2 = bias_pair(beta2)
    bx3, br3 = bias_pair(beta3)
    return dict(
        w1t=w1t, w2t=w2t, w3t=w3t, w4t=w4t,
        bx1=bx1, br1=br1, bx2=bx2, br2=br2, bx3=bx3, br3=br3,
    ), ec


def kernel(species, aev, W1, b1, W2, b2, W3, b3, W4, b4):
    global LAST_EXEC_NS
    species = np.asarray(species)
    aev = np.asarray(aev, dtype=np.float32)
    args = [np.asarray(x, dtype=np.float32)
            for x in (W1, b1, W2, b2, W3, b3, W4, b4)]
    wp, ec = _prep_weights(*args)

    # --- host routing: per-core counting sort by species ---
    sp_c = species.reshape(NCORES, BM * A)
    counts = np.stack([np.bincount(sp_c[c], minlength=S) for c in range(NCORES)])
    caps = tuple(
        int(((counts[:, s].max() + 31) // 32) * 32) for s in range(S)
    )
    offs = np.concatenate([[0], np.cumsum(caps)]).astype(np.int64)
    a_pad = int(offs[-1])

    if caps not in _CACHE:
        _CACHE[caps] = _build(caps)
    nc = _CACHE[caps]

    aev_c = aev.reshape(NCORES, BM * A, F)
    in_maps = []
    perms = []
    for c in range(NCORES):
        perm = np.argsort(sp_c[c], kind="stable")
        perms.append(perm)
        xt = np.zeros((F, a_pad), np.float32)
        pos = 0
        for s in range(S):
            n = counts[c, s]
            xt[:, offs[s] : offs[s] + n] = aev_c[c][perm[pos : pos + n]].T
            pos += n
        m = {
            "xt": xt,
            "w1t": wp["w1t"], "w2t": wp["w2t"], "w3t": wp["w3t"], "w4t": wp["w4t"],
            "bx1": wp["bx1"], "br1": wp["br1"],
            "bx2": wp["bx2"], "br2": wp["br2"],
            "bx3": wp["bx3"], "br3": wp["br3"],
        }
        in_maps.append(m)

    trace = bool(os.environ.get("KERNEL_TRACE"))
    res = run_bass_kernel_spmd(
        nc, in_maps, list(range(NCORES)), trace=trace
    )
    LAST_EXEC_NS = res.exec_time_ns

    # --- host reduction: scatter atom energies back to molecules ---
    out = np.zeros((NCORES, BM), np.float64)
    for c in range(NCORES):
        en = np.asarray(res.results[c]["energy"][0], np.float64)
        atom_e = np.empty(BM * A, np.float64)
        pos = 0
        for s in range(S):
            n = counts[c, s]
            atom_e[perms[c][pos : pos + n]] = en[offs[s] : offs[s] + n]
            pos += n
        out[c] = atom_e.reshape(BM, A).sum(axis=1)
        out[c] += np.asarray(ec, np.float64)[sp_c[c]].reshape(BM, A).sum(axis=1)
    return out.reshape(B).astype(np.float32)


# revision 6
# speedup vs baseline: 1.5546x; 1.5546x over previous
"""ANI-style species-routed MLP (MoE routing) on 8 TRN2 NeuronCores.

Strategy:
- Data-parallel over molecules: core c handles molecules [128c, 128(c+1)).
- Host groups each core's 6144 atoms by species (counting sort), pads each
  species bucket to a shared uniform capacity, and ships the aev
  feature-major (transposed) so features sit on SBUF partitions.
- Device computes, per species segment, the dense 4-layer MLP chain with
  f32r matmuls (full PE rate), CELU via one exact trick:
      celu(x) + 0.1 = min(0.1*exp(10x), 0.1) + relu(x)
  The +0.1 offset is folded into the next layer's bias on the host
  (beta = b - 0.1 * rowsum(W)); the 0.1 output scale of the exp is folded
  into the activation bias (ln(0.1)).
- Final per-molecule reduction on host (cheap), including the per-species
  output bias b4 - 0.1*rowsum(W4).
"""
import os
import sys

sys.path.insert(0, "/opt/trn_rl_repo")

from contextlib import ExitStack

import numpy as np

import concourse.bass as bass
import concourse.mybir as mybir
import concourse.tile as tile
from concourse import bacc
from concourse.bass_utils import run_bass_kernel_spmd

F32 = mybir.dt.float32
F32R = mybir.dt.float32r
AF = mybir.ActivationFunctionType
ALU = mybir.AluOpType

B, A, F = 1024, 48, 384
S = 7
NCORES = 8
BM = B // NCORES  # molecules per core
ALPHA = 0.1
LN_ALPHA = float(np.log(ALPHA))

_CACHE = {}
LAST_EXEC_NS = None

# which celu chunks put relu on ACT instead of DVE (load balance): 1-in-3
RELU_ACT_MOD, RELU_ACT_PHASE = 3, 2


def _build(cap):
    """SPMD graph: uniform per-species capacity `cap` (atoms, mult of 64)."""
    half = cap // 2
    a_pad = S * cap
    nc = bacc.Bacc()

    xt_d = nc.declare_dram_parameter("xt", [F, a_pad], F32, isOutput=False)
    w1_d = nc.declare_dram_parameter("w1t", [S, 384, 256], F32, isOutput=False)
    w2_d = nc.declare_dram_parameter("w2t", [S, 256, 192], F32, isOutput=False)
    w3_d = nc.declare_dram_parameter("w3t", [S, 256, 160], F32, isOutput=False)
    w4_d = nc.declare_dram_parameter("w4t", [S, 256, 1], F32, isOutput=False)
    # biases: [species, layer(3), kind(exp/relu), chunk(2), 128]
    b_d = nc.declare_dram_parameter("biases", [S, 3, 2, 2, 128], F32, isOutput=False)
    en_d = nc.declare_dram_parameter("energy", [1, a_pad], F32, isOutput=True)

    l1_m = [(0, 128), (128, 128)]
    l2_m = [(0, 128), (128, 64)]
    l3_m = [(0, 128), (128, 32)]
    l2_k = [(0, 128), (128, 128)]
    l3_k = [(0, 128), (128, 64)]
    l4_k = [(0, 128), (128, 32)]

    with tile.TileContext(nc) as tc, ExitStack() as ctx:
        wpool = ctx.enter_context(tc.tile_pool(name="weights", bufs=1))
        xpool = ctx.enter_context(tc.tile_pool(name="x", bufs=4))
        upool = ctx.enter_context(tc.tile_pool(name="u", bufs=2))
        tpool = ctx.enter_context(tc.tile_pool(name="t", bufs=3))
        zpool = ctx.enter_context(tc.tile_pool(name="z", bufs=3, space="PSUM"))
        z4pool = ctx.enter_context(tc.tile_pool(name="z4", bufs=2, space="PSUM"))
        epool = ctx.enter_context(tc.tile_pool(name="en", bufs=1))

        w1 = wpool.tile([128, S, 3, 256], F32R)
        nc.sync.dma_start(
            w1[:], w1_d.ap().bitcast(F32R).rearrange("s (c p) m -> p s c m", p=128)
        )
        w2 = wpool.tile([128, S, 2, 192], F32R)
        nc.sync.dma_start(
            w2[:], w2_d.ap().bitcast(F32R).rearrange("s (c p) m -> p s c m", p=128)
        )
        w3 = wpool.tile([128, S, 2, 160], F32R)
        nc.sync.dma_start(
            w3[:], w3_d.ap().bitcast(F32R).rearrange("s (c p) m -> p s c m", p=128)
        )
        w4 = wpool.tile([128, S, 2, 1], F32R)
        nc.sync.dma_start(
            w4[:], w4_d.ap().bitcast(F32R).rearrange("s (c p) m -> p s c m", p=128)
        )
        bb = wpool.tile([128, S, 3, 2, 2], F32)
        nc.sync.dma_start(bb[:], b_d.ap().rearrange("s l k c p -> p s l k c"))

        en_sb = epool.tile([1, a_pad], F32)

        chunk_idx = 0

        def celu(z, u_out, s, l, m):
            nonlocal chunk_idx
            p = z.shape[0]
            n = z.shape[-1]
            bx = bb[:p, s, l - 1, 0, m : m + 1]
            br = bb[:p, s, l - 1, 1, m : m + 1]
            e = tpool.tile([128, cap], F32, tag="e")
            nc.scalar.activation(e[:p, :n], z[:], AF.Exp, bias=bx, scale=10.0)
            r = tpool.tile([128, cap], F32, tag="r")
            if chunk_idx % RELU_ACT_MOD == RELU_ACT_PHASE:
                nc.scalar.activation(r[:p, :n], z[:], AF.Relu, bias=br, scale=1.0)
            else:
                nc.vector.tensor_scalar(
                    r[:p, :n], z[:], br, 0.0, op0=ALU.add, op1=ALU.max
                )
            chunk_idx += 1
            nc.vector.scalar_tensor_tensor(
                u_out[:], e[:p, :n], ALPHA, r[:p, :n], op0=ALU.min, op1=ALU.add
            )

        for s in range(S):
            x = xpool.tile([128, 3, cap], F32R, tag="x")
            nc.sync.dma_start(
                x[:],
                xt_d.ap()[:, s * cap : (s + 1) * cap]
                .bitcast(F32R)
                .rearrange("(c p) n -> p c n", p=128),
            )
            u1 = upool.tile([128, 2, cap], F32R, tag="u1")
            for mi, (mo, mw) in enumerate(l1_m):
                z = zpool.tile([128, cap], F32, tag="z")
                for h in range(2):
                    hs = slice(h * half, (h + 1) * half)
                    for k in range(3):
                        nc.tensor.matmul(
                            z[:mw, hs],
                            w1[:, s, k, mo : mo + mw],
                            x[:, k, hs],
                            start=(k == 0),
                            stop=(k == 2),
                        )
                celu(z[:mw, :], u1[:mw, mi, :], s, 1, mi)
            u2 = upool.tile([128, 2, cap], F32R, tag="u2")
            for mi, (mo, mw) in enumerate(l2_m):
                z = zpool.tile([128, cap], F32, tag="z")
                for h in range(2):
                    hs = slice(h * half, (h + 1) * half)
                    for ki, (ko, kw) in enumerate(l2_k):
                        nc.tensor.matmul(
                            z[:mw, hs],
                            w2[:kw, s, ki, mo : mo + mw],
                            u1[:kw, ki, hs],
                            start=(ki == 0),
                            stop=(ki == 1),
                        )
                celu(z[:mw, :], u2[:mw, mi, :], s, 2, mi)
            u3 = upool.tile([128, 2, cap], F32R, tag="u3")
            for mi, (mo, mw) in enumerate(l3_m):
                z = zpool.tile([128, cap], F32, tag="z")
                for h in range(2):
                    hs = slice(h * half, (h + 1) * half)
                    for ki, (ko, kw) in enumerate(l3_k):
                        nc.tensor.matmul(
                            z[:mw, hs],
                            w3[:kw, s, ki, mo : mo + mw],
                            u2[:kw, ki, hs],
                            start=(ki == 0),
                            stop=(ki == 1),
                        )
                celu(z[:mw, :], u3[:mw, mi, :], s, 3, mi)
            for h in range(2):
                hs = slice(h * half, (h + 1) * half)
                row = 2 * s + h
                z4 = z4pool.tile([1, half], F32, tag="z4")
                for ki, (ko, kw) in enumerate(l4_k):
                    nc.tensor.matmul(
                        z4[:],
                        w4[:kw, s, ki, 0:1],
                        u3[:kw, ki, hs],
                        start=(ki == 0),
                        stop=(ki == 1),
                    )
                oo = row * half
                if row % 2 == 0:
                    nc.vector.tensor_copy(en_sb[0:1, oo : oo + half], z4[:])
                else:
                    nc.scalar.activation(en_sb[0:1, oo : oo + half], z4[:], AF.Copy)

        nc.sync.dma_start(en_d.ap(), en_sb[:])

    nc.compile()
    return nc


def _prep_weights(W1, b1, W2, b2, W3, b3, W4, b4):
    beta1 = b1
    beta2 = b2 - ALPHA * W2.sum(axis=2)
    beta3 = b3 - ALPHA * W3.sum(axis=2)
    ec = (b4[:, 0] - ALPHA * W4[:, 0, :].sum(axis=1)).astype(np.float32)

    def pad_k(wt, k_to):  # wt [S, K, M] -> [S, k_to, M]
        out = np.zeros((S, k_to, wt.shape[2]), np.float32)
        out[:, : wt.shape[1]] = wt
        return out

    w1t = np.ascontiguousarray(W1.transpose(0, 2, 1))  # [S, 384, 256]
    w2t = np.ascontiguousarray(W2.transpose(0, 2, 1))  # [S, 256, 192]
    w3t = pad_k(W3.transpose(0, 2, 1), 256)  # [S, 256, 160]
    w4t = pad_k(W4.transpose(0, 2, 1), 256)  # [S, 256, 1]

    biases = np.zeros((S, 3, 2, 2, 128), np.float32)
    for li, beta in enumerate((beta1, beta2, beta3)):
        m = beta.shape[1]
        bx = np.zeros((S, 256), np.float32)
        br = np.zeros((S, 256), np.float32)
        bx[:, :m] = 10.0 * beta + LN_ALPHA
        br[:, :m] = beta
        biases[:, li, 0] = bx.reshape(S, 2, 128)
        biases[:, li, 1] = br.reshape(S, 2, 128)
    return dict(w1t=w1t, w2t=w2t, w3t=w3t, w4t=w4t, biases=biases), ec


def kernel(species, aev, W1, b1, W2, b2, W3, b3, W4, b4):
    global LAST_EXEC_NS
    species = np.asarray(species)
    aev = np.asarray(aev, dtype=np.float32)
    args = [np.asarray(x, dtype=np.float32)
            for x in (W1, b1, W2, b2, W3, b3, W4, b4)]
    wp, ec = _prep_weights(*args)

    # --- host routing: per-core counting sort by species ---
    sp_c = species.reshape(NCORES, BM * A)
    counts = np.stack([np.bincount(sp_c[c], minlength=S) for c in range(NCORES)])
    cap = int(((counts.max() + 63) // 64) * 64)
    cap = max(cap, 128)
    a_pad = S * cap

    if cap not in _CACHE:
        _CACHE[cap] = _build(cap)
    nc = _CACHE[cap]

    aev_c = aev.reshape(NCORES, BM * A, F)
    in_maps = []
    perms = []
    for c in range(NCORES):
        perm = np.argsort(sp_c[c], kind="stable")
        perms.append(perm)
        xt = np.zeros((F, a_pad), np.float32)
        pos = 0
        for s in range(S):
            n = counts[c, s]
            xt[:, s * cap : s * cap + n] = aev_c[c][perm[pos : pos + n]].T
            pos += n
        m = {"xt": xt, **wp}
        in_maps.append(m)

    trace = bool(os.environ.get("KERNEL_TRACE"))
    res = run_bass_kernel_spmd(nc, in_maps, list(range(NCORES)), trace=trace)
    LAST_EXEC_NS = res.exec_time_ns

    # --- host reduction: scatter atom energies back to molecules ---
    out = np.zeros((NCORES, BM), np.float64)
    for c in range(NCORES):
        en = np.asarray(res.results[c]["energy"][0], np.float64)
        atom_e = np.empty(BM * A, np.float64)
        pos = 0
        for s in range(S):
            n = counts[c, s]
            atom_e[perms[c][pos : pos + n]] = en[s * cap : s * cap + n]
            pos += n
        out[c] = atom_e.reshape(BM, A).sum(axis=1)
        out[c] += np.asarray(ec, np.float64)[sp_c[c]].reshape(BM, A).sum(axis=1)
    return out.reshape(B).astype(np.float32)
